# revision 20
# baseline (speedup 1.0000x reference)
"""MiniMax Lightning Attention kernel for 8 TRN2 NeuronCores (v2).

Data-parallel over 8192 tokens (1024/core). Per core:
  - q/k projection in fp8-e4m3 DoubleRow matmuls (inputs scaled x16,
    PSUM rescaled /256); v projection in bf16.
  - partial RoPE + (elu+1) feature map on q/k (bf16 vector ops).
  - per-token head-mixing attention: S[b,n,j] = q'.k', attn = (S/norm)@v
    (normalizer ksum is AllReduced across cores, 4x GQA factor folded
    into w_o).
  - o_proj in bf16 with w_o as the stationary operand -> transposed
    output outT[ocol, tok]; the host transposes back after gather.
Phases are arranged so o_proj (PE) overlaps attention (DVE) via a
5-tile/3-tile split of the token tiles.
"""
import sys
sys.path.insert(0, "/opt/trn_rl_repo")

import numpy as np
import ml_dtypes

import concourse.bass as bass
import concourse.bacc as bacc
import concourse.mybir as mybir
import concourse.tile as tile
from concourse import masks
from concourse.bass_utils import run_bass_kernel_spmd

F32 = mybir.dt.float32
BF16 = mybir.dt.bfloat16
FP8 = mybir.dt.float8e4
ALU = mybir.AluOpType
AF = mybir.ActivationFunctionType
DR = mybir.MatmulPerfMode.DoubleRow
ts = bass.ts

# problem shape (hardcoded per contest contract)
B = 8192
HID = 4096
NH = 32
NKV = 8
D = 128
ROT = 64
HALF = 32
ROPE_BASE = 10000000.0

NCORES = 8
BC = B // NCORES           # 1024 tokens per core
P = 128
TT = BC // P               # 8 token tiles per core
KC = HID // P              # 32 contraction chunks of 128
C8 = HID // 256            # 16 contraction chunks of 256 (DoubleRow)
QCOLS = NH * D             # 4096
KCOLS = NKV * D            # 1024
QKCOLS = QCOLS + KCOLS     # 5120
FSCL = 16.0                # fp8 input scale
PSCL = 1.0 / (FSCL * FSCL)  # PSUM rescale

TA = 5                     # token tiles in first attention/o_proj half
TB = TT - TA

_CACHE: dict = {}


def _emit_rope_elu(nc, pools, x, cs, sn, nh):
    """In-place partial rope + elu+1 on x: [128, nh, 128] bf16 slice."""
    t1 = pools["rope"].tile([P, nh, ROT], BF16, tag=f"r1_{nh}")
    t2 = pools["rope"].tile([P, nh, ROT], BF16, tag=f"r2_{nh}")
    csb = cs[:].unsqueeze(1).broadcast_to([P, nh, ROT])
    snb = sn[:].unsqueeze(1).broadcast_to([P, nh, ROT])
    xr = x[:, :, 0:ROT]
    nc.vector.tensor_mul(t1[:], xr, csb)
    nc.vector.tensor_mul(t2[:], xr, snb)
    nc.vector.tensor_sub(x[:, :, 0:HALF], t1[:, :, 0:HALF], t2[:, :, HALF:ROT])
    nc.vector.tensor_add(x[:, :, HALF:ROT], t1[:, :, HALF:ROT], t2[:, :, 0:HALF])
    # elu+1: f(x) = min(exp(x),1) + max(x,0)
    flat = x.rearrange("p n d -> p (n d)")
    e = pools["elu"].tile([P, nh * D], BF16, tag=f"e_{nh}")
    nc.scalar.activation(e[:], flat, AF.Exp)
    nc.vector.tensor_scalar_min(e[:], e[:], 1.0)
    nc.vector.scalar_tensor_tensor(flat, flat, 0.0, e[:], op0=ALU.max, op1=ALU.add)


def _build():
    nc = bacc.Bacc("TRN2", target_bir_lowering=False, debug=False,
                   enable_asserts=False, num_devices=NCORES)

    hT8 = nc.dram_tensor("hT8", [C8, P, 2, BC], FP8, kind="ExternalInput").ap()
    w8 = nc.dram_tensor("w8", [C8, P, 2, QKCOLS], FP8, kind="ExternalInput").ap()
    hTv = nc.dram_tensor("hTv", [2, KC, P, 512], BF16, kind="ExternalInput").ap()
    wv = nc.dram_tensor("wv", [2, KC, P, 512], BF16, kind="ExternalInput").ap()
    wot = nc.dram_tensor("wot", [KC, 8, P, 512], BF16, kind="ExternalInput").ap()
    csb = nc.dram_tensor("csb", [TT, P, ROT], BF16, kind="ExternalInput").ap()
    snb = nc.dram_tensor("snb", [TT, P, ROT], BF16, kind="ExternalInput").ap()
    outT = nc.dram_tensor("outT", [HID, BC], BF16, kind="ExternalOutput").ap()

    with tile.TileContext(nc) as tc:
        with tc.tile_pool(name="res", bufs=1) as res, \
             tc.tile_pool(name="qfeat", bufs=1) as qfeatp, \
             tc.tile_pool(name="small", bufs=2) as small, \
             tc.tile_pool(name="attn", bufs=3) as attnp, \
             tc.tile_pool(name="outsb", bufs=2) as outsb, \
             tc.tile_pool(name="tpps", bufs=2, space="PSUM") as tpps, \
             tc.tile_pool(name="warmp", bufs=1, space="PSUM") as warmp, \
             tc.tile_pool(name="dram", bufs=1, space="DRAM") as dram:

            # ---------------- global residents ----------------
            ident = res.tile([P, P], BF16, tag="ident")
            masks.make_identity(nc, ident[:])
            ones_b = res.tile([P, 1], BF16, tag="ones")
            nc.vector.memset(ones_b[:], 1.0)
            warm_ps = warmp.tile([1, 8], F32, tag="warm")

            cs_sb, sn_sb = [], []
            for t in range(TT):
                ct = res.tile([P, ROT], BF16, tag=f"cs{t}")
                st = res.tile([P, ROT], BF16, tag=f"sn{t}")
                nc.sync.dma_start(ct[:], csb[t])
                nc.sync.dma_start(st[:], snb[t])
                cs_sb.append(ct)
                sn_sb.append(st)

            qf01p = tc.alloc_tile_pool(name="qf01", bufs=1, side="right")
            aT01p = tc.alloc_tile_pool(name="aT01p", bufs=1, side="right")
            kb = [res.tile([P, NKV, D], BF16, tag=f"kb{t}", name=f"kb{t}")
                  for t in range(TT)]
            vb = [res.tile([P, NKV, D], BF16, tag=f"vb{t}", name=f"vb{t}")
                  for t in range(TT)]
            ksum_rep = res.tile([P, NKV, D], BF16, tag="ksum_rep")
            qf = [(qf01p if t < 2 else qfeatp).tile(
                      [P, NH, D], BF16, tag=f"qf{t}", name=f"qf{t}")
                  for t in range(TT)]
            # staging for attnT columns of tiles 0-1 (attention runs in
            # block 1, before the attnT pool exists)
            aT01 = [aT01p.tile([P, 2 * P], BF16, tag=f"a01_{ch}",
                               name=f"a01_{ch}") for ch in range(KC)]

            def emit_warm(src_ap):
                # Tiny matmul reading a tile the DVE just wrote: keeps the
                # PE activity monitor from seeing an idle window (which
                # would clock-gate the whole core to half rate).
                nc.tensor.matmul(warm_ps[:], ones_b[:], src_ap,
                                 start=True, stop=True,
                                 skip_group_check=True)

            def emit_attention_g(t, g, dst, dcol):
                """Attention for 4 heads of group g, token tile t.
                dst[ch] tiles receive transposed attn at col dcol."""
                qsl = qf[t][:, g * 4:(g + 1) * 4, :]
                normt = small.tile([P, 4], F32, tag="norm")
                for h in range(4):
                    scr = small.tile([P, D], BF16, tag="nscr")
                    nc.vector.scalar_tensor_tensor(
                        scr[:], qsl[:, h, :], 1.0, ksum_rep[:, g, :],
                        op0=ALU.mult, op1=ALU.mult,
                        accum_out=normt[:, h:h + 1])
                nc.vector.tensor_scalar_add(normt[:], normt[:], 1e-6)
                rn = small.tile([P, 4], F32, tag="rn")
                nc.vector.reciprocal(rn[:], normt[:])
                for h in range(4):
                    nc.vector.tensor_scalar_mul(
                        qsl[:, h, :], qsl[:, h, :], rn[:, h:h + 1])
                Sf = small.tile([P, 4 * NKV], F32, tag="Sf")
                for h in range(4):
                    for j in range(NKV):
                        scr = small.tile([P, D], BF16, tag="sscr")
                        nc.vector.scalar_tensor_tensor(
                            scr[:], qsl[:, h, :], 1.0, kb[t][:, j, :],
                            op0=ALU.mult, op1=ALU.mult,
                            accum_out=Sf[:, h * NKV + j:h * NKV + j + 1])
                    emit_warm(scr[:, 0:8])
                attn_g = attnp.tile([P, 4, D], BF16, tag="attn")
                for h in range(4):
                    nc.vector.tensor_scalar_mul(
                        attn_g[:, h, :], vb[t][:, 0, :],
                        Sf[:, h * NKV:h * NKV + 1])
                    for j in range(1, NKV):
                        nc.vector.scalar_tensor_tensor(
                            attn_g[:, h, :], vb[t][:, j, :],
                            Sf[:, h * NKV + j:h * NKV + j + 1],
                            attn_g[:, h, :],
                            op0=ALU.mult, op1=ALU.add)
                    emit_warm(attn_g[:, h, 0:8])
                tpt = tpps.tile([P, 4, D], BF16, tag="tp")
                for h in range(4):
                    nc.tensor.transpose(
                        tpt[:, h, :], attn_g[:, h, :], ident[:])
                for h in range(4):
                    nc.scalar.activation(
                        dst[g * 4 + h][:, dcol:dcol + P], tpt[:, h, :],
                        AF.Copy)

            # ============ scoped block 1: projections ============
            with tc.tile_pool(name="h8p", bufs=1) as h8p, \
                 tc.tile_pool(name="w8s", bufs=20) as w8s, \
                 tc.tile_pool(name="hvs", bufs=4) as hvs, \
                 tc.tile_pool(name="wvs", bufs=4) as wvs, \
                 tc.tile_pool(name="ropet", bufs=2) as ropet, \
                 tc.tile_pool(name="elut", bufs=2) as elut, \
                 tc.tile_pool(name="gemm", bufs=4, space="PSUM") as gemm:

                pools = {"rope": ropet, "elu": elut}

                hT8_sb = []
                for c in range(C8):
                    h8 = h8p.tile([P, 2, BC], FP8, tag=f"h8_{c}")
                    nc.sync.dma_start(h8[:], hT8[c])
                    hT8_sb.append(h8)

                with tc.tile_pool(name="ksps", bufs=1, space="PSUM") as ksps:
                    # ---- phase 1: k projection (fp8 DoubleRow) ----
                    for gk in range(2):
                        col0 = QCOLS + gk * 512
                        w_t = []
                        for c in range(C8):
                            wt = w8s.tile([P, 2, 512], FP8, tag="w8")
                            nc.sync.dma_start(wt[:], w8[c][:, :, col0:col0 + 512])
                            w_t.append(wt)
                        for t in range(TT):
                            ps = gemm.tile([P, 512], F32, tag="mm")
                            for c in range(C8):
                                nc.tensor.matmul(
                                    ps[:], hT8_sb[c][:, :, ts(t, P)], w_t[c][:],
                                    perf_mode=DR, start=(c == 0),
                                    stop=(c == C8 - 1))
                            ksl = kb[t][:, gk * 4:(gk + 1) * 4, :]
                            nc.scalar.activation(
                                ksl.rearrange("p n d -> p (n d)"), ps[:],
                                AF.Copy, scale=PSCL)
                            _emit_rope_elu(nc, pools, ksl, cs_sb[t], sn_sb[t], 4)

                    # ---- phase 2: ksum + AllReduce (async vs phase 3) ----
                    ks_sb = h8p.tile([1, NKV * D], F32, tag="kssb")
                    for half in range(2):
                        ks_ps = ksps.tile([1, 512], F32, tag="ks")
                        for t in range(TT):
                            nc.tensor.matmul(
                                ks_ps[:], ones_b[:],
                                kb[t][:, half * 4:(half + 1) * 4, :],
                                start=(t == 0), stop=(t == TT - 1))
                        nc.vector.tensor_copy(ks_sb[0:1, ts(half, 512)],
                                              ks_ps[:])
                    ks_in = dram.tile([1, NKV * D], F32)
                    ks_out = dram.tile([1, NKV * D], F32)
                    nc.sync.dma_start(ks_in[:], ks_sb[:])
                    nc.gpsimd.collective_compute(
                        "AllReduce", ALU.add,
                        replica_groups=[list(range(NCORES))],
                        ins=[ks_in[:].opt()],
                        outs=[ks_out[:].opt()],
                    )

                # ---- phase 3: v projection (bf16) ----
                for vg in range(2):
                    for th in range(2):
                        ps_v = [gemm.tile([P, 512], F32, tag="mm",
                                          name=f"psv{vg}{th}{i}")
                                for i in range(4)]
                        hv_t, wv_t = [], []
                        for kc in range(KC):
                            hv = hvs.tile([P, 512], BF16, tag="hv")
                            nc.sync.dma_start(hv[:], hTv[th, kc])
                            wvt = wvs.tile([P, 512], BF16, tag="wv")
                            nc.sync.dma_start(wvt[:], wv[vg, kc])
                            hv_t.append(hv)
                            wv_t.append(wvt)
                        for kc in range(KC):
                            for tq in range(4):
                                nc.tensor.matmul(
                                    ps_v[tq][:], hv_t[kc][:, ts(tq, P)],
                                    wv_t[kc][:],
                                    start=(kc == 0), stop=(kc == KC - 1))
                        for tq in range(4):
                            t = th * 4 + tq
                            nc.scalar.activation(
                                vb[t][:, vg * 4:(vg + 1) * 4, :]
                                .rearrange("p n d -> p (n d)"),
                                ps_v[tq][:], AF.Copy)

                # ---- phase 4: broadcast allreduced ksum ----
                ksf32 = h8p.tile([P, NKV * D], F32, tag="ksf32")
                nc.sync.dma_start(ksf32[:], ks_out[:].broadcast_to([P, NKV * D]))
                nc.vector.tensor_copy(
                    ksum_rep[:].rearrange("p n d -> p (n d)"), ksf32[:])

                # ---- phase 5a: q projection + attention for tiles 0-1 ----
                for g in range(NH // 4):
                    col0 = g * 512
                    w_t = []
                    for c in range(C8):
                        wt = w8s.tile([P, 2, 512], FP8, tag="w8")
                        nc.sync.dma_start(wt[:], w8[c][:, :, col0:col0 + 512])
                        w_t.append(wt)
                    for t in range(TT):
                        ps = gemm.tile([P, 512], F32, tag="mm")
                        for c in range(C8):
                            nc.tensor.matmul(
                                ps[:], hT8_sb[c][:, :, ts(t, P)], w_t[c][:],
                                perf_mode=DR, start=(c == 0), stop=(c == C8 - 1))
                        qsl = qf[t][:, g * 4:(g + 1) * 4, :]
                        nc.scalar.activation(
                            qsl.rearrange("p n d -> p (n d)"), ps[:],
                            AF.Copy, scale=PSCL)
                        _emit_rope_elu(nc, pools, qsl, cs_sb[t], sn_sb[t], 4)
                    # attention for tiles 0-1 overlaps the next group's GEMMs
                    emit_attention_g(0, g, aT01, 0)
                    emit_attention_g(1, g, aT01, P)

            # ============ scoped block 2: attention + o_proj ============
            with tc.tile_pool(name="attnT", bufs=1) as attnTp:

                attnT = [attnTp.tile([P, BC], BF16, tag=f"aT{ch}",
                                     name=f"aT{ch}") for ch in range(KC)]
                for ch in range(KC):
                    nc.scalar.activation(attnT[ch][:, 0:2 * P], aT01[ch][:],
                                         AF.Copy)
                with tc.tile_pool(name="wos", bufs=18) as wos, \
                     tc.tile_pool(name="poTp", bufs=5, space="PSUM") as poTp:

                    nchunk = [0]

                    def emit_oproj_chunk(t0, ntile):
                        """One ocg chunk (512 out cols) of o_proj for a
                        token range; ~20us of PE work."""
                        ocg = nchunk[0] % 8
                        eng = nc.sync if nchunk[0] % 2 == 0 else nc.scalar
                        nchunk[0] += 1
                        ntok = ntile * P
                        poTs = [poTp.tile([P, ntok], F32, tag="poT",
                                          name=f"poT{nchunk[0]}{i}")
                                for i in range(4)]
                        for kh in range(2):
                            w_t = []
                            for kc in range(16 * kh, 16 * kh + 16):
                                wt = wos.tile([P, 512], BF16, tag="wo")
                                eng.dma_start(wt[:], wot[kc, ocg])
                                w_t.append(wt)
                            for ocq in range(4):
                                for i, kc in enumerate(
                                        range(16 * kh, 16 * kh + 16)):
                                    nc.tensor.matmul(
                                        poTs[ocq][:], w_t[i][:, ts(ocq, P)],
                                        attnT[kc][:, t0 * P:t0 * P + ntok],
                                        start=(kc == 0), stop=(kc == KC - 1))
                        for ocq in range(4):
                            oc = ocg * 4 + ocq
                            ot = outsb.tile([P, 384], BF16, tag="ot")
                            nc.scalar.activation(ot[:, 0:ntok], poTs[ocq][:],
                                                 AF.Copy)
                            nc.sync.dma_start(
                                outT[ts(oc, P), t0 * P:t0 * P + ntok],
                                ot[:, 0:ntok])

                    # attention t2-7, woven with o_proj sweeps over done
                    # token ranges: sweep1 = t0-1, sweep2 = t2-4,
                    # sweep3 = t5-7.
                    for t in (2, 3, 4):
                        for g in range(NH // 4):
                            emit_attention_g(t, g, attnT, t * P)
                            if ((t - 2) * 8 + g) % 3 == 2:
                                emit_oproj_chunk(0, 2)
                    for t in (5, 6):
                        for g in range(NH // 4):
                            emit_attention_g(t, g, attnT, t * P)
                            if ((t - 5) * 8 + g) % 2 == 1:
                                emit_oproj_chunk(2, 3)
                    for g in range(NH // 4):
                        emit_attention_g(7, g, attnT, 7 * P)
                        emit_oproj_chunk(5, 2)
                    for _ in range(8):
                        emit_oproj_chunk(7, 1)

            aT01p.release()
            qf01p.release()

    nc.compile()
    return nc


def _get_nc():
    if "nc" not in _CACHE:
        _CACHE["nc"] = _build()
    return _CACHE["nc"]


def kernel(hidden_states, positions, w_qkv, w_o):
    nc = _get_nc()

    bf16 = ml_dtypes.bfloat16
    fp8 = ml_dtypes.float8_e4m3

    wqkvT = np.ascontiguousarray(w_qkv.astype(np.float32).T)  # [4096, 6144]
    w8_np = (wqkvT[:, :QKCOLS] * np.float32(FSCL)).reshape(
        C8, 2, P, QKCOLS).transpose(0, 2, 1, 3)
    w8_np = np.ascontiguousarray(w8_np).astype(fp8)
    wv_np = wqkvT[:, QKCOLS:].reshape(KC, P, 2, 512).transpose(2, 0, 1, 3)
    wv_np = np.ascontiguousarray(wv_np).astype(bf16)
    woT4 = np.ascontiguousarray(w_o.astype(np.float32).T) * np.float32(4.0)
    wot_np = woT4.reshape(KC, P, 8, 512).transpose(0, 2, 1, 3)
    wot_np = np.ascontiguousarray(wot_np).astype(bf16)

    pos_f = positions.astype(np.float32)
    k = np.arange(0, ROT, 2, dtype=np.float32)
    inv_freq = (np.float32(1.0) /
                np.power(np.float32(ROPE_BASE), k / np.float32(ROT)))
    freqs = pos_f[:, None] * inv_freq[None, :].astype(np.float32)
    cos = np.cos(freqs).astype(np.float32)
    sin = np.sin(freqs).astype(np.float32)
    cs = np.concatenate([cos, cos], axis=1).astype(bf16)  # [8192, 64]
    sn = np.concatenate([sin, sin], axis=1).astype(bf16)

    hT_full = np.ascontiguousarray(hidden_states.astype(np.float32).T)

    in_maps = []
    for c in range(NCORES):
        sl = slice(c * BC, (c + 1) * BC)
        hT = np.ascontiguousarray(hT_full[:, sl])  # [4096, 1024]
        h8 = (hT * np.float32(FSCL)).reshape(C8, 2, P, BC).transpose(0, 2, 1, 3)
        h8 = np.ascontiguousarray(h8).astype(fp8)
        hv = hT.reshape(KC, P, 2, 512).transpose(2, 0, 1, 3)
        hv = np.ascontiguousarray(hv).astype(bf16)
        in_maps.append({
            "hT8": h8,
            "w8": w8_np,
            "hTv": hv,
            "wv": wv_np,
            "wot": wot_np,
            "csb": np.ascontiguousarray(cs[sl].reshape(TT, P, ROT)),
            "snb": np.ascontiguousarray(sn[sl].reshape(TT, P, ROT)),
        })

    res = run_bass_kernel_spmd(nc, in_maps, core_ids=list(range(NCORES)),
                               **_CACHE.get("run_kwargs", {}))
    _CACHE["last_result"] = res
    return np.concatenate(
        [np.ascontiguousarray(
            res.results[c]["outT"].astype(np.float32).T)
         for c in range(NCORES)],
        axis=0)


# revision 21
# speedup vs baseline: 1.0062x; 1.0062x over previous
"""MiniMax Lightning Attention kernel for 8 TRN2 NeuronCores (v2).

Data-parallel over 8192 tokens (1024/core). Per core:
  - q/k projection in fp8-e4m3 DoubleRow matmuls (inputs scaled x16,
    PSUM rescaled /256); v projection in bf16.
  - partial RoPE + (elu+1) feature map on q/k (bf16 vector ops).
  - per-token head-mixing attention: S[b,n,j] = q'.k', attn = (S/norm)@v
    (normalizer ksum is AllReduced across cores, 4x GQA factor folded
    into w_o).
  - o_proj in bf16 with w_o as the stationary operand -> transposed
    output outT[ocol, tok]; the host transposes back after gather.
Phases are arranged so o_proj (PE) overlaps attention (DVE) via a
5-tile/3-tile split of the token tiles.
"""
import sys
sys.path.insert(0, "/opt/trn_rl_repo")

import numpy as np
import ml_dtypes

import concourse.bass as bass
import concourse.bacc as bacc
import concourse.mybir as mybir
import concourse.tile as tile
from concourse import masks
from concourse.bass_utils import run_bass_kernel_spmd

F32 = mybir.dt.float32
BF16 = mybir.dt.bfloat16
FP8 = mybir.dt.float8e4
ALU = mybir.AluOpType
AF = mybir.ActivationFunctionType
DR = mybir.MatmulPerfMode.DoubleRow
ts = bass.ts

# problem shape (hardcoded per contest contract)
B = 8192
HID = 4096
NH = 32
NKV = 8
D = 128
ROT = 64
HALF = 32
ROPE_BASE = 10000000.0

NCORES = 8
BC = B // NCORES           # 1024 tokens per core
P = 128
TT = BC // P               # 8 token tiles per core
KC = HID // P              # 32 contraction chunks of 128
C8 = HID // 256            # 16 contraction chunks of 256 (DoubleRow)
QCOLS = NH * D             # 4096
KCOLS = NKV * D            # 1024
QKCOLS = QCOLS + KCOLS     # 5120
FSCL = 16.0                # fp8 input scale
PSCL = 1.0 / (FSCL * FSCL)  # PSUM rescale

TA = 5                     # token tiles in first attention/o_proj half
TB = TT - TA

_CACHE: dict = {}


def _emit_rope_elu(nc, pools, x, cs, sn, nh):
    """In-place partial rope + elu+1 on x: [128, nh, 128] bf16 slice."""
    t1 = pools["rope"].tile([P, nh, ROT], BF16, tag=f"r1_{nh}")
    t2 = pools["rope"].tile([P, nh, ROT], BF16, tag=f"r2_{nh}")
    csb = cs[:].unsqueeze(1).broadcast_to([P, nh, ROT])
    snb = sn[:].unsqueeze(1).broadcast_to([P, nh, ROT])
    xr = x[:, :, 0:ROT]
    nc.vector.tensor_mul(t1[:], xr, csb)
    nc.vector.tensor_mul(t2[:], xr, snb)
    nc.vector.tensor_sub(x[:, :, 0:HALF], t1[:, :, 0:HALF], t2[:, :, HALF:ROT])
    nc.vector.tensor_add(x[:, :, HALF:ROT], t1[:, :, HALF:ROT], t2[:, :, 0:HALF])
    # elu+1: f(x) = min(exp(x),1) + max(x,0)
    flat = x.rearrange("p n d -> p (n d)")
    e = pools["elu"].tile([P, nh * D], BF16, tag=f"e_{nh}")
    nc.scalar.activation(e[:], flat, AF.Exp)
    nc.vector.tensor_scalar_min(e[:], e[:], 1.0)
    nc.vector.scalar_tensor_tensor(flat, flat, 0.0, e[:], op0=ALU.max, op1=ALU.add)


def _build():
    nc = bacc.Bacc("TRN2", target_bir_lowering=False, debug=False,
                   enable_asserts=False, num_devices=NCORES)

    hT8 = nc.dram_tensor("hT8", [C8, P, 2, BC], FP8, kind="ExternalInput").ap()
    w8 = nc.dram_tensor("w8", [C8, P, 2, QKCOLS], FP8, kind="ExternalInput").ap()
    hTv = nc.dram_tensor("hTv", [2, KC, P, 512], BF16, kind="ExternalInput").ap()
    wv = nc.dram_tensor("wv", [2, KC, P, 512], BF16, kind="ExternalInput").ap()
    wot = nc.dram_tensor("wot", [KC, 8, P, 512], BF16, kind="ExternalInput").ap()
    csb = nc.dram_tensor("csb", [TT, P, ROT], BF16, kind="ExternalInput").ap()
    snb = nc.dram_tensor("snb", [TT, P, ROT], BF16, kind="ExternalInput").ap()
    outT = nc.dram_tensor("outT", [HID, BC], BF16, kind="ExternalOutput").ap()

    with tile.TileContext(nc) as tc:
        with tc.tile_pool(name="res", bufs=1) as res, \
             tc.tile_pool(name="qfeat", bufs=1) as qfeatp, \
             tc.tile_pool(name="small", bufs=2) as small, \
             tc.tile_pool(name="ropet", bufs=2) as ropet, \
             tc.tile_pool(name="elut", bufs=2) as elut, \
             tc.tile_pool(name="attn", bufs=2) as attnp, \
             tc.tile_pool(name="outsb", bufs=2) as outsb, \
             tc.tile_pool(name="tpps", bufs=2, space="PSUM") as tpps, \
             tc.tile_pool(name="warmp", bufs=1, space="PSUM") as warmp, \
             tc.tile_pool(name="dram", bufs=1, space="DRAM") as dram:

            # ---------------- global residents ----------------
            ident = res.tile([P, P], BF16, tag="ident")
            masks.make_identity(nc, ident[:])
            ones_b = res.tile([P, 1], BF16, tag="ones")
            nc.vector.memset(ones_b[:], 1.0)
            warm_ps = warmp.tile([1, 8], F32, tag="warm")

            cs_sb, sn_sb = [], []
            for t in range(TT):
                ct = res.tile([P, ROT], BF16, tag=f"cs{t}")
                st = res.tile([P, ROT], BF16, tag=f"sn{t}")
                nc.sync.dma_start(ct[:], csb[t])
                nc.sync.dma_start(st[:], snb[t])
                cs_sb.append(ct)
                sn_sb.append(st)

            qf01p = tc.alloc_tile_pool(name="qf01", bufs=1, side="right")
            aT01p = tc.alloc_tile_pool(name="aT01p", bufs=1, side="right")
            kb = [res.tile([P, NKV, D], BF16, tag=f"kb{t}", name=f"kb{t}")
                  for t in range(TT)]
            vb = [res.tile([P, NKV, D], BF16, tag=f"vb{t}", name=f"vb{t}")
                  for t in range(TT)]
            ksum_rep = res.tile([P, NKV, D], BF16, tag="ksum_rep")
            qf = [(qf01p if t < 2 else qfeatp).tile(
                      [P, NH, D], BF16, tag=f"qf{t}", name=f"qf{t}")
                  for t in range(TT)]
            # staging for attnT columns of tiles 0-1 (attention runs in
            # block 1, before the attnT pool exists)
            aT01 = [aT01p.tile([P, 2 * P], BF16, tag=f"a01_{ch}",
                               name=f"a01_{ch}") for ch in range(KC)]

            def emit_warm(src_ap):
                # Tiny matmul reading a tile the DVE just wrote: keeps the
                # PE activity monitor from seeing an idle window (which
                # would clock-gate the whole core to half rate).
                nc.tensor.matmul(warm_ps[:], ones_b[:], src_ap,
                                 start=True, stop=True,
                                 skip_group_check=True)

            def emit_attention_g(t, g, dst, dcol):
                """Attention for 4 heads of group g, token tile t.
                dst[ch] tiles receive transposed attn at col dcol."""
                qsl = qf[t][:, g * 4:(g + 1) * 4, :]
                normt = small.tile([P, 4], F32, tag="norm")
                for h in range(4):
                    scr = small.tile([P, D], BF16, tag="nscr")
                    nc.vector.scalar_tensor_tensor(
                        scr[:], qsl[:, h, :], 1.0, ksum_rep[:, g, :],
                        op0=ALU.mult, op1=ALU.mult,
                        accum_out=normt[:, h:h + 1])
                nc.vector.tensor_scalar_add(normt[:], normt[:], 1e-6)
                rn = small.tile([P, 4], F32, tag="rn")
                nc.vector.reciprocal(rn[:], normt[:])
                for h in range(4):
                    nc.vector.tensor_scalar_mul(
                        qsl[:, h, :], qsl[:, h, :], rn[:, h:h + 1])
                Sf = small.tile([P, 4 * NKV], F32, tag="Sf")
                for h in range(4):
                    for j in range(NKV):
                        scr = small.tile([P, D], BF16, tag="sscr")
                        nc.vector.scalar_tensor_tensor(
                            scr[:], qsl[:, h, :], 1.0, kb[t][:, j, :],
                            op0=ALU.mult, op1=ALU.mult,
                            accum_out=Sf[:, h * NKV + j:h * NKV + j + 1])
                    emit_warm(scr[:, 0:8])
                attn_g = attnp.tile([P, 4, D], BF16, tag="attn")
                for h in range(4):
                    nc.vector.tensor_scalar_mul(
                        attn_g[:, h, :], vb[t][:, 0, :],
                        Sf[:, h * NKV:h * NKV + 1])
                    for j in range(1, NKV):
                        nc.vector.scalar_tensor_tensor(
                            attn_g[:, h, :], vb[t][:, j, :],
                            Sf[:, h * NKV + j:h * NKV + j + 1],
                            attn_g[:, h, :],
                            op0=ALU.mult, op1=ALU.add)
                    emit_warm(attn_g[:, h, 0:8])
                tpt = tpps.tile([P, 4, D], BF16, tag="tp")
                for h in range(4):
                    nc.tensor.transpose(
                        tpt[:, h, :], attn_g[:, h, :], ident[:])
                for h in range(4):
                    nc.scalar.activation(
                        dst[g * 4 + h][:, dcol:dcol + P], tpt[:, h, :],
                        AF.Copy)

            # ============ scoped block 1: projections ============
            with tc.tile_pool(name="h8p", bufs=1) as h8p, \
                 tc.tile_pool(name="w8s", bufs=20) as w8s, \
                 tc.tile_pool(name="hvs", bufs=4) as hvs, \
                 tc.tile_pool(name="wvs", bufs=4) as wvs, \
                 tc.tile_pool(name="gemm", bufs=4, space="PSUM") as gemm:

                pools = {"rope": ropet, "elu": elut}

                hT8_sb = []
                for c in range(C8):
                    h8 = h8p.tile([P, 2, BC], FP8, tag=f"h8_{c}")
                    nc.sync.dma_start(h8[:], hT8[c])
                    hT8_sb.append(h8)

                with tc.tile_pool(name="ksps", bufs=1, space="PSUM") as ksps:
                    # ---- phase 1: k projection (fp8 DoubleRow) ----
                    for gk in range(2):
                        col0 = QCOLS + gk * 512
                        w_t = []
                        for c in range(C8):
                            wt = w8s.tile([P, 2, 512], FP8, tag="w8")
                            nc.sync.dma_start(wt[:], w8[c][:, :, col0:col0 + 512])
                            w_t.append(wt)
                        for t in range(TT):
                            ps = gemm.tile([P, 512], F32, tag="mm")
                            for c in range(C8):
                                nc.tensor.matmul(
                                    ps[:], hT8_sb[c][:, :, ts(t, P)], w_t[c][:],
                                    perf_mode=DR, start=(c == 0),
                                    stop=(c == C8 - 1))
                            ksl = kb[t][:, gk * 4:(gk + 1) * 4, :]
                            nc.scalar.activation(
                                ksl.rearrange("p n d -> p (n d)"), ps[:],
                                AF.Copy, scale=PSCL)
                            _emit_rope_elu(nc, pools, ksl, cs_sb[t], sn_sb[t], 4)

                    # ---- phase 2: ksum + AllReduce (async vs phase 3) ----
                    ks_sb = h8p.tile([1, NKV * D], F32, tag="kssb")
                    for half in range(2):
                        ks_ps = ksps.tile([1, 512], F32, tag="ks")
                        for t in range(TT):
                            nc.tensor.matmul(
                                ks_ps[:], ones_b[:],
                                kb[t][:, half * 4:(half + 1) * 4, :],
                                start=(t == 0), stop=(t == TT - 1))
                        nc.vector.tensor_copy(ks_sb[0:1, ts(half, 512)],
                                              ks_ps[:])
                    ks_in = dram.tile([1, NKV * D], F32)
                    ks_out = dram.tile([1, NKV * D], F32)
                    nc.sync.dma_start(ks_in[:], ks_sb[:])
                    nc.gpsimd.collective_compute(
                        "AllReduce", ALU.add,
                        replica_groups=[list(range(NCORES))],
                        ins=[ks_in[:].opt()],
                        outs=[ks_out[:].opt()],
                    )

                # ---- phase 3: v projection (bf16) ----
                for vg in range(2):
                    for th in range(2):
                        ps_v = [gemm.tile([P, 512], F32, tag="mm",
                                          name=f"psv{vg}{th}{i}")
                                for i in range(4)]
                        hv_t, wv_t = [], []
                        for kc in range(KC):
                            hv = hvs.tile([P, 512], BF16, tag="hv")
                            nc.sync.dma_start(hv[:], hTv[th, kc])
                            wvt = wvs.tile([P, 512], BF16, tag="wv")
                            nc.sync.dma_start(wvt[:], wv[vg, kc])
                            hv_t.append(hv)
                            wv_t.append(wvt)
                        for kc in range(KC):
                            for tq in range(4):
                                nc.tensor.matmul(
                                    ps_v[tq][:], hv_t[kc][:, ts(tq, P)],
                                    wv_t[kc][:],
                                    start=(kc == 0), stop=(kc == KC - 1))
                        for tq in range(4):
                            t = th * 4 + tq
                            nc.scalar.activation(
                                vb[t][:, vg * 4:(vg + 1) * 4, :]
                                .rearrange("p n d -> p (n d)"),
                                ps_v[tq][:], AF.Copy)

                # ---- phase 4: broadcast allreduced ksum ----
                ksf32 = h8p.tile([P, NKV * D], F32, tag="ksf32")
                nc.sync.dma_start(ksf32[:], ks_out[:].broadcast_to([P, NKV * D]))
                nc.vector.tensor_copy(
                    ksum_rep[:].rearrange("p n d -> p (n d)"), ksf32[:])

                # ---- phase 5a: q projection + attention for tiles 0-1 ----
                for g in range(NH // 4):
                    col0 = g * 512
                    w_t = []
                    for c in range(C8):
                        wt = w8s.tile([P, 2, 512], FP8, tag="w8")
                        nc.sync.dma_start(wt[:], w8[c][:, :, col0:col0 + 512])
                        w_t.append(wt)
                    for t in range(TT):
                        ps = gemm.tile([P, 512], F32, tag="mm")
                        for c in range(C8):
                            nc.tensor.matmul(
                                ps[:], hT8_sb[c][:, :, ts(t, P)], w_t[c][:],
                                perf_mode=DR, start=(c == 0), stop=(c == C8 - 1))
                        qsl = qf[t][:, g * 4:(g + 1) * 4, :]
                        nc.scalar.activation(
                            qsl.rearrange("p n d -> p (n d)"), ps[:],
                            AF.Copy, scale=PSCL)
                        _emit_rope_elu(nc, pools, qsl, cs_sb[t], sn_sb[t], 4)
                    # attention for tiles 0-1 overlaps the next group's GEMMs
                    emit_attention_g(0, g, aT01, 0)
                    emit_attention_g(1, g, aT01, P)

            # ============ scoped block 2: attention + o_proj ============
            with tc.tile_pool(name="attnT", bufs=1) as attnTp:

                attnT = [attnTp.tile([P, BC], BF16, tag=f"aT{ch}",
                                     name=f"aT{ch}") for ch in range(KC)]
                for ch in range(KC):
                    nc.scalar.activation(attnT[ch][:, 0:2 * P], aT01[ch][:],
                                         AF.Copy)
                with tc.tile_pool(name="wos", bufs=18) as wos, \
                     tc.tile_pool(name="poTp", bufs=5, space="PSUM") as poTp:

                    nchunk = [0]

                    def emit_oproj_chunk(t0, ntile):
                        """One ocg chunk (512 out cols) of o_proj for a
                        token range; ~20us of PE work."""
                        ocg = nchunk[0] % 8
                        eng = nc.sync if nchunk[0] % 2 == 0 else nc.scalar
                        nchunk[0] += 1
                        ntok = ntile * P
                        poTs = [poTp.tile([P, ntok], F32, tag="poT",
                                          name=f"poT{nchunk[0]}{i}")
                                for i in range(4)]
                        for kh in range(2):
                            w_t = []
                            for kc in range(16 * kh, 16 * kh + 16):
                                wt = wos.tile([P, 512], BF16, tag="wo")
                                eng.dma_start(wt[:], wot[kc, ocg])
                                w_t.append(wt)
                            for ocq in range(4):
                                for i, kc in enumerate(
                                        range(16 * kh, 16 * kh + 16)):
                                    nc.tensor.matmul(
                                        poTs[ocq][:], w_t[i][:, ts(ocq, P)],
                                        attnT[kc][:, t0 * P:t0 * P + ntok],
                                        start=(kc == 0), stop=(kc == KC - 1))
                        for ocq in range(4):
                            oc = ocg * 4 + ocq
                            ot = outsb.tile([P, 384], BF16, tag="ot")
                            nc.scalar.activation(ot[:, 0:ntok], poTs[ocq][:],
                                                 AF.Copy)
                            nc.sync.dma_start(
                                outT[ts(oc, P), t0 * P:t0 * P + ntok],
                                ot[:, 0:ntok])

                    # attention t2-7, woven with o_proj sweeps over done
                    # token ranges: sweep1 = t0-1, sweep2 = t2-4,
                    # sweep3 = t5-7.
                    for t in (2, 3, 4):
                        for g in range(NH // 4):
                            emit_attention_g(t, g, attnT, t * P)
                            if ((t - 2) * 8 + g) % 3 == 2:
                                emit_oproj_chunk(0, 2)
                    for t in (5, 6):
                        for g in range(NH // 4):
                            emit_attention_g(t, g, attnT, t * P)
                            if ((t - 5) * 8 + g) % 2 == 1:
                                emit_oproj_chunk(2, 3)
                    for g in range(NH // 4):
                        emit_attention_g(7, g, attnT, 7 * P)
                        emit_oproj_chunk(5, 2)
                    for _ in range(8):
                        emit_oproj_chunk(7, 1)

            aT01p.release()
            qf01p.release()

    nc.compile()
    return nc


def _get_nc():
    if "nc" not in _CACHE:
        _CACHE["nc"] = _build()
    return _CACHE["nc"]


def kernel(hidden_states, positions, w_qkv, w_o):
    nc = _get_nc()

    bf16 = ml_dtypes.bfloat16
    fp8 = ml_dtypes.float8_e4m3

    wqkvT = np.ascontiguousarray(w_qkv.astype(np.float32).T)  # [4096, 6144]
    w8_np = (wqkvT[:, :QKCOLS] * np.float32(FSCL)).reshape(
        C8, 2, P, QKCOLS).transpose(0, 2, 1, 3)
    w8_np = np.ascontiguousarray(w8_np).astype(fp8)
    wv_np = wqkvT[:, QKCOLS:].reshape(KC, P, 2, 512).transpose(2, 0, 1, 3)
    wv_np = np.ascontiguousarray(wv_np).astype(bf16)
    woT4 = np.ascontiguousarray(w_o.astype(np.float32).T) * np.float32(4.0)
    wot_np = woT4.reshape(KC, P, 8, 512).transpose(0, 2, 1, 3)
    wot_np = np.ascontiguousarray(wot_np).astype(bf16)

    pos_f = positions.astype(np.float32)
    k = np.arange(0, ROT, 2, dtype=np.float32)
    inv_freq = (np.float32(1.0) /
                np.power(np.float32(ROPE_BASE), k / np.float32(ROT)))
    freqs = pos_f[:, None] * inv_freq[None, :].astype(np.float32)
    cos = np.cos(freqs).astype(np.float32)
    sin = np.sin(freqs).astype(np.float32)
    cs = np.concatenate([cos, cos], axis=1).astype(bf16)  # [8192, 64]
    sn = np.concatenate([sin, sin], axis=1).astype(bf16)

    hT_full = np.ascontiguousarray(hidden_states.astype(np.float32).T)

    in_maps = []
    for c in range(NCORES):
        sl = slice(c * BC, (c + 1) * BC)
        hT = np.ascontiguousarray(hT_full[:, sl])  # [4096, 1024]
        h8 = (hT * np.float32(FSCL)).reshape(C8, 2, P, BC).transpose(0, 2, 1, 3)
        h8 = np.ascontiguousarray(h8).astype(fp8)
        hv = hT.reshape(KC, P, 2, 512).transpose(2, 0, 1, 3)
        hv = np.ascontiguousarray(hv).astype(bf16)
        in_maps.append({
            "hT8": h8,
            "w8": w8_np,
            "hTv": hv,
            "wv": wv_np,
            "wot": wot_np,
            "csb": np.ascontiguousarray(cs[sl].reshape(TT, P, ROT)),
            "snb": np.ascontiguousarray(sn[sl].reshape(TT, P, ROT)),
        })

    res = run_bass_kernel_spmd(nc, in_maps, core_ids=list(range(NCORES)),
                               **_CACHE.get("run_kwargs", {}))
    _CACHE["last_result"] = res
    return np.concatenate(
        [np.ascontiguousarray(
            res.results[c]["outT"].astype(np.float32).T)
         for c in range(NCORES)],
        axis=0)


# revision 22
# speedup vs baseline: 1.0665x; 1.0600x over previous
"""MiniMax Lightning Attention kernel for 8 TRN2 NeuronCores (v2).

Data-parallel over 8192 tokens (1024/core). Per core:
  - q/k projection in fp8-e4m3 DoubleRow matmuls (inputs scaled x16,
    PSUM rescaled /256); v projection in bf16.
  - partial RoPE + (elu+1) feature map on q/k (bf16 vector ops).
  - per-token head-mixing attention: S[b,n,j] = q'.k', attn = (S/norm)@v
    (normalizer ksum is AllReduced across cores, 4x GQA factor folded
    into w_o).
  - o_proj in bf16 with w_o as the stationary operand -> transposed
    output outT[ocol, tok]; the host transposes back after gather.
Phases are arranged so o_proj (PE) overlaps attention (DVE) via a
5-tile/3-tile split of the token tiles.
"""
import sys
sys.path.insert(0, "/opt/trn_rl_repo")

import numpy as np
import ml_dtypes

import concourse.bass as bass
import concourse.bacc as bacc
import concourse.mybir as mybir
import concourse.tile as tile
from concourse import masks
from concourse.bass_utils import run_bass_kernel_spmd

F32 = mybir.dt.float32
BF16 = mybir.dt.bfloat16
FP8 = mybir.dt.float8e4
ALU = mybir.AluOpType
AF = mybir.ActivationFunctionType
DR = mybir.MatmulPerfMode.DoubleRow
ts = bass.ts

# problem shape (hardcoded per contest contract)
B = 8192
HID = 4096
NH = 32
NKV = 8
D = 128
ROT = 64
HALF = 32
ROPE_BASE = 10000000.0

NCORES = 8
BC = B // NCORES           # 1024 tokens per core
P = 128
TT = BC // P               # 8 token tiles per core
KC = HID // P              # 32 contraction chunks of 128
C8 = HID // 256            # 16 contraction chunks of 256 (DoubleRow)
QCOLS = NH * D             # 4096
KCOLS = NKV * D            # 1024
QKCOLS = QCOLS + KCOLS     # 5120
FSCL = 16.0                # fp8 input scale
PSCL = 1.0 / (FSCL * FSCL)  # PSUM rescale

TA = 5                     # token tiles in first attention/o_proj half
TB = TT - TA

_CACHE: dict = {}


def _emit_rope_elu(nc, pools, x, cs, sn, nh):
    """In-place partial rope + elu+1 on x: [128, nh, 128] bf16 slice."""
    t1 = pools["rope"].tile([P, nh, ROT], BF16, tag=f"r1_{nh}")
    t2 = pools["rope"].tile([P, nh, ROT], BF16, tag=f"r2_{nh}")
    csb = cs[:].unsqueeze(1).broadcast_to([P, nh, ROT])
    snb = sn[:].unsqueeze(1).broadcast_to([P, nh, ROT])
    xr = x[:, :, 0:ROT]
    nc.vector.tensor_mul(t1[:], xr, csb)
    nc.vector.tensor_mul(t2[:], xr, snb)
    nc.vector.tensor_sub(x[:, :, 0:HALF], t1[:, :, 0:HALF], t2[:, :, HALF:ROT])
    nc.vector.tensor_add(x[:, :, HALF:ROT], t1[:, :, HALF:ROT], t2[:, :, 0:HALF])
    # elu+1: f(x) = min(exp(x),1) + max(x,0)
    flat = x.rearrange("p n d -> p (n d)")
    e = pools["elu"].tile([P, nh * D], BF16, tag=f"e_{nh}")
    nc.scalar.activation(e[:], flat, AF.Exp)
    nc.vector.tensor_scalar_min(e[:], e[:], 1.0)
    nc.vector.scalar_tensor_tensor(flat, flat, 0.0, e[:], op0=ALU.max, op1=ALU.add)


def _build():
    nc = bacc.Bacc("TRN2", target_bir_lowering=False, debug=False,
                   enable_asserts=False, num_devices=NCORES)

    hT8 = nc.dram_tensor("hT8", [C8, P, 2, BC], FP8, kind="ExternalInput").ap()
    w8 = nc.dram_tensor("w8", [C8, P, 2, QKCOLS], FP8, kind="ExternalInput").ap()
    hTv = nc.dram_tensor("hTv", [2, KC, P, 512], BF16, kind="ExternalInput").ap()
    wv = nc.dram_tensor("wv", [2, KC, P, 512], BF16, kind="ExternalInput").ap()
    wot = nc.dram_tensor("wot", [KC, 8, P, 512], BF16, kind="ExternalInput").ap()
    csb = nc.dram_tensor("csb", [TT, P, ROT], BF16, kind="ExternalInput").ap()
    snb = nc.dram_tensor("snb", [TT, P, ROT], BF16, kind="ExternalInput").ap()
    outT = nc.dram_tensor("outT", [HID, BC], BF16, kind="ExternalOutput").ap()

    with tile.TileContext(nc) as tc:
        with tc.tile_pool(name="res", bufs=1) as res, \
             tc.tile_pool(name="qfeat", bufs=1) as qfeatp, \
             tc.tile_pool(name="small", bufs=2) as small, \
             tc.tile_pool(name="ropet", bufs=2) as ropet, \
             tc.tile_pool(name="elut", bufs=2) as elut, \
             tc.tile_pool(name="attn", bufs=2) as attnp, \
             tc.tile_pool(name="outsb", bufs=2) as outsb, \
             tc.tile_pool(name="tpps", bufs=2, space="PSUM") as tpps, \
             tc.tile_pool(name="warmp", bufs=1, space="PSUM") as warmp, \
             tc.tile_pool(name="dram", bufs=1, space="DRAM") as dram:

            # ---------------- global residents ----------------
            ident = res.tile([P, P], BF16, tag="ident")
            masks.make_identity(nc, ident[:])
            ones_b = res.tile([P, 1], BF16, tag="ones")
            nc.vector.memset(ones_b[:], 1.0)
            warm_ps = warmp.tile([1, 8], F32, tag="warm")

            cs_sb, sn_sb = [], []
            for t in range(TT):
                ct = res.tile([P, ROT], BF16, tag=f"cs{t}")
                st = res.tile([P, ROT], BF16, tag=f"sn{t}")
                nc.sync.dma_start(ct[:], csb[t])
                nc.sync.dma_start(st[:], snb[t])
                cs_sb.append(ct)
                sn_sb.append(st)

            qf01p = tc.alloc_tile_pool(name="qf01", bufs=1, side="right")
            aT01p = tc.alloc_tile_pool(name="aT01p", bufs=1, side="right")
            kb = [res.tile([P, NKV, D], BF16, tag=f"kb{t}", name=f"kb{t}")
                  for t in range(TT)]
            vb = [res.tile([P, NKV, D], BF16, tag=f"vb{t}", name=f"vb{t}")
                  for t in range(TT)]
            ksum_rep = res.tile([P, NKV, D], BF16, tag="ksum_rep")
            qf = [(qf01p if t < 2 else qfeatp).tile(
                      [P, NH, D], BF16, tag=f"qf{t}", name=f"qf{t}")
                  for t in range(TT)]
            # staging for attnT columns of tiles 0-1 (attention runs in
            # block 1, before the attnT pool exists)
            aT01 = [aT01p.tile([P, 2 * P], BF16, tag=f"a01_{ch}",
                               name=f"a01_{ch}") for ch in range(KC)]

            def emit_warm(src_ap):
                # Tiny matmul reading a tile the DVE just wrote: keeps the
                # PE activity monitor from seeing an idle window (which
                # would clock-gate the whole core to half rate).
                nc.tensor.matmul(warm_ps[:], ones_b[:], src_ap,
                                 start=True, stop=True,
                                 skip_group_check=True)

            def emit_attention_g(t, g, dst, dcol, warm=True):
                """Attention for 4 heads of group g, token tile t.
                dst[ch] tiles receive transposed attn at col dcol."""
                qsl = qf[t][:, g * 4:(g + 1) * 4, :]
                normt = small.tile([P, 4], F32, tag="norm")
                for h in range(4):
                    scr = small.tile([P, D], BF16, tag="nscr")
                    nc.vector.scalar_tensor_tensor(
                        scr[:], qsl[:, h, :], 1.0, ksum_rep[:, g, :],
                        op0=ALU.mult, op1=ALU.mult,
                        accum_out=normt[:, h:h + 1])
                nc.vector.tensor_scalar_add(normt[:], normt[:], 1e-6)
                rn = small.tile([P, 4], F32, tag="rn")
                nc.vector.reciprocal(rn[:], normt[:])
                for h in range(4):
                    nc.vector.tensor_scalar_mul(
                        qsl[:, h, :], qsl[:, h, :], rn[:, h:h + 1])
                Sf = small.tile([P, 4 * NKV], F32, tag="Sf")
                for h in range(4):
                    for j in range(NKV):
                        scr = small.tile([P, D], BF16, tag="sscr", bufs=4)
                        nc.vector.scalar_tensor_tensor(
                            scr[:], qsl[:, h, :], 1.0, kb[t][:, j, :],
                            op0=ALU.mult, op1=ALU.mult,
                            accum_out=Sf[:, h * NKV + j:h * NKV + j + 1])
                    if warm and h == 3:
                        emit_warm(scr[:, 0:8])
                attn_g = attnp.tile([P, 4, D], BF16, tag="attn")
                for h in range(4):
                    nc.vector.tensor_scalar_mul(
                        attn_g[:, h, :], vb[t][:, 0, :],
                        Sf[:, h * NKV:h * NKV + 1])
                    for j in range(1, NKV):
                        nc.vector.scalar_tensor_tensor(
                            attn_g[:, h, :], vb[t][:, j, :],
                            Sf[:, h * NKV + j:h * NKV + j + 1],
                            attn_g[:, h, :],
                            op0=ALU.mult, op1=ALU.add)
                    if warm and h == 3:
                        emit_warm(attn_g[:, h, 0:8])
                tpt = tpps.tile([P, 4, D], BF16, tag="tp")
                for h in range(4):
                    nc.tensor.transpose(
                        tpt[:, h, :], attn_g[:, h, :], ident[:])
                for h in range(4):
                    nc.scalar.activation(
                        dst[g * 4 + h][:, dcol:dcol + P], tpt[:, h, :],
                        AF.Copy)

            # ============ scoped block 1: projections ============
            with tc.tile_pool(name="h8p", bufs=1) as h8p, \
                 tc.tile_pool(name="w8s", bufs=20) as w8s, \
                 tc.tile_pool(name="hvs", bufs=4) as hvs, \
                 tc.tile_pool(name="wvs", bufs=4) as wvs, \
                 tc.tile_pool(name="gemm", bufs=4, space="PSUM") as gemm:

                pools = {"rope": ropet, "elu": elut}

                hT8_sb = []
                for c in range(C8):
                    h8 = h8p.tile([P, 2, BC], FP8, tag=f"h8_{c}")
                    nc.sync.dma_start(h8[:], hT8[c])
                    hT8_sb.append(h8)

                with tc.tile_pool(name="ksps", bufs=1, space="PSUM") as ksps:
                    # ---- phase 1: k projection (fp8 DoubleRow) ----
                    for gk in range(2):
                        col0 = QCOLS + gk * 512
                        w_t = []
                        for c in range(C8):
                            wt = w8s.tile([P, 2, 512], FP8, tag="w8")
                            nc.sync.dma_start(wt[:], w8[c][:, :, col0:col0 + 512])
                            w_t.append(wt)
                        for t in range(TT):
                            ps = gemm.tile([P, 512], F32, tag="mm")
                            for c in range(C8):
                                nc.tensor.matmul(
                                    ps[:], hT8_sb[c][:, :, ts(t, P)], w_t[c][:],
                                    perf_mode=DR, start=(c == 0),
                                    stop=(c == C8 - 1))
                            ksl = kb[t][:, gk * 4:(gk + 1) * 4, :]
                            nc.scalar.activation(
                                ksl.rearrange("p n d -> p (n d)"), ps[:],
                                AF.Copy, scale=PSCL)
                            _emit_rope_elu(nc, pools, ksl, cs_sb[t], sn_sb[t], 4)

                    # ---- phase 2: ksum + AllReduce (async vs phase 3) ----
                    ks_sb = h8p.tile([1, NKV * D], F32, tag="kssb")
                    for half in range(2):
                        ks_ps = ksps.tile([1, 512], F32, tag="ks")
                        for t in range(TT):
                            nc.tensor.matmul(
                                ks_ps[:], ones_b[:],
                                kb[t][:, half * 4:(half + 1) * 4, :],
                                start=(t == 0), stop=(t == TT - 1))
                        nc.vector.tensor_copy(ks_sb[0:1, ts(half, 512)],
                                              ks_ps[:])
                    ks_in = dram.tile([1, NKV * D], F32)
                    ks_out = dram.tile([1, NKV * D], F32)
                    nc.sync.dma_start(ks_in[:], ks_sb[:])
                    nc.gpsimd.collective_compute(
                        "AllReduce", ALU.add,
                        replica_groups=[list(range(NCORES))],
                        ins=[ks_in[:].opt()],
                        outs=[ks_out[:].opt()],
                    )

                # ---- phase 3: v projection (bf16) ----
                for vg in range(2):
                    for th in range(2):
                        ps_v = [gemm.tile([P, 512], F32, tag="mm",
                                          name=f"psv{vg}{th}{i}")
                                for i in range(4)]
                        hv_t, wv_t = [], []
                        for kc in range(KC):
                            hv = hvs.tile([P, 512], BF16, tag="hv")
                            nc.sync.dma_start(hv[:], hTv[th, kc])
                            wvt = wvs.tile([P, 512], BF16, tag="wv")
                            nc.sync.dma_start(wvt[:], wv[vg, kc])
                            hv_t.append(hv)
                            wv_t.append(wvt)
                        for kc in range(KC):
                            for tq in range(4):
                                nc.tensor.matmul(
                                    ps_v[tq][:], hv_t[kc][:, ts(tq, P)],
                                    wv_t[kc][:],
                                    start=(kc == 0), stop=(kc == KC - 1))
                        for tq in range(4):
                            t = th * 4 + tq
                            nc.scalar.activation(
                                vb[t][:, vg * 4:(vg + 1) * 4, :]
                                .rearrange("p n d -> p (n d)"),
                                ps_v[tq][:], AF.Copy)

                # ---- phase 4: broadcast allreduced ksum ----
                ksf32 = h8p.tile([P, NKV * D], F32, tag="ksf32")
                nc.sync.dma_start(ksf32[:], ks_out[:].broadcast_to([P, NKV * D]))
                nc.vector.tensor_copy(
                    ksum_rep[:].rearrange("p n d -> p (n d)"), ksf32[:])

                # ---- phase 5a: q projection + attention for tiles 0-1 ----
                for g in range(NH // 4):
                    col0 = g * 512
                    w_t = []
                    for c in range(C8):
                        wt = w8s.tile([P, 2, 512], FP8, tag="w8")
                        nc.sync.dma_start(wt[:], w8[c][:, :, col0:col0 + 512])
                        w_t.append(wt)
                    for t in range(TT):
                        ps = gemm.tile([P, 512], F32, tag="mm")
                        for c in range(C8):
                            nc.tensor.matmul(
                                ps[:], hT8_sb[c][:, :, ts(t, P)], w_t[c][:],
                                perf_mode=DR, start=(c == 0), stop=(c == C8 - 1))
                        qsl = qf[t][:, g * 4:(g + 1) * 4, :]
                        nc.scalar.activation(
                            qsl.rearrange("p n d -> p (n d)"), ps[:],
                            AF.Copy, scale=PSCL)
                        _emit_rope_elu(nc, pools, qsl, cs_sb[t], sn_sb[t], 4)
                    # attention for tiles 0-1 overlaps the next group's GEMMs
                    emit_attention_g(0, g, aT01, 0, warm=False)
                    emit_attention_g(1, g, aT01, P, warm=False)

            # ============ scoped block 2: attention + o_proj ============
            with tc.tile_pool(name="attnT", bufs=1) as attnTp:

                attnT = [attnTp.tile([P, BC], BF16, tag=f"aT{ch}",
                                     name=f"aT{ch}") for ch in range(KC)]
                for ch in range(KC):
                    nc.scalar.activation(attnT[ch][:, 0:2 * P], aT01[ch][:],
                                         AF.Copy)
                with tc.tile_pool(name="wos", bufs=18) as wos, \
                     tc.tile_pool(name="poTp", bufs=5, space="PSUM") as poTp:

                    nchunk = [0]

                    def emit_oproj_chunk(t0, ntile):
                        """One ocg chunk (512 out cols) of o_proj for a
                        token range; ~20us of PE work."""
                        ocg = nchunk[0] % 8
                        eng = nc.sync if nchunk[0] % 2 == 0 else nc.scalar
                        nchunk[0] += 1
                        ntok = ntile * P
                        poTs = [poTp.tile([P, ntok], F32, tag="poT",
                                          name=f"poT{nchunk[0]}{i}")
                                for i in range(4)]
                        for kh in range(2):
                            w_t = []
                            for kc in range(16 * kh, 16 * kh + 16):
                                wt = wos.tile([P, 512], BF16, tag="wo")
                                eng.dma_start(wt[:], wot[kc, ocg])
                                w_t.append(wt)
                            for ocq in range(4):
                                for i, kc in enumerate(
                                        range(16 * kh, 16 * kh + 16)):
                                    nc.tensor.matmul(
                                        poTs[ocq][:], w_t[i][:, ts(ocq, P)],
                                        attnT[kc][:, t0 * P:t0 * P + ntok],
                                        start=(kc == 0), stop=(kc == KC - 1))
                        for ocq in range(4):
                            oc = ocg * 4 + ocq
                            ot = outsb.tile([P, 384], BF16, tag="ot")
                            nc.scalar.activation(ot[:, 0:ntok], poTs[ocq][:],
                                                 AF.Copy)
                            nc.sync.dma_start(
                                outT[ts(oc, P), t0 * P:t0 * P + ntok],
                                ot[:, 0:ntok])

                    # attention t2-7, woven with pair-tile o_proj sweeps
                    # one window behind: t0-1 into t2-3, t2-3 into t4-5,
                    # t4-5 into t6-7, t6-7 as the tail.
                    for t in (2, 3):
                        for g in range(NH // 4):
                            emit_attention_g(t, g, attnT, t * P)
                            if ((t - 2) * 8 + g) % 2 == 1:
                                emit_oproj_chunk(0, 2)
                    for t in (4, 5):
                        for g in range(NH // 4):
                            emit_attention_g(t, g, attnT, t * P)
                            if ((t - 4) * 8 + g) % 2 == 1:
                                emit_oproj_chunk(2, 2)
                    for t in (6, 7):
                        for g in range(NH // 4):
                            emit_attention_g(t, g, attnT, t * P)
                            if ((t - 6) * 8 + g) % 2 == 1:
                                emit_oproj_chunk(4, 2)
                    for _ in range(8):
                        emit_oproj_chunk(6, 2)

            aT01p.release()
            qf01p.release()

    nc.compile()
    return nc


def _get_nc():
    if "nc" not in _CACHE:
        _CACHE["nc"] = _build()
    return _CACHE["nc"]


def kernel(hidden_states, positions, w_qkv, w_o):
    nc = _get_nc()

    bf16 = ml_dtypes.bfloat16
    fp8 = ml_dtypes.float8_e4m3

    wqkvT = np.ascontiguousarray(w_qkv.astype(np.float32).T)  # [4096, 6144]
    w8_np = (wqkvT[:, :QKCOLS] * np.float32(FSCL)).reshape(
        C8, 2, P, QKCOLS).transpose(0, 2, 1, 3)
    w8_np = np.ascontiguousarray(w8_np).astype(fp8)
    wv_np = wqkvT[:, QKCOLS:].reshape(KC, P, 2, 512).transpose(2, 0, 1, 3)
    wv_np = np.ascontiguousarray(wv_np).astype(bf16)
    woT4 = np.ascontiguousarray(w_o.astype(np.float32).T) * np.float32(4.0)
    wot_np = woT4.reshape(KC, P, 8, 512).transpose(0, 2, 1, 3)
    wot_np = np.ascontiguousarray(wot_np).astype(bf16)

    pos_f = positions.astype(np.float32)
    k = np.arange(0, ROT, 2, dtype=np.float32)
    inv_freq = (np.float32(1.0) /
                np.power(np.float32(ROPE_BASE), k / np.float32(ROT)))
    freqs = pos_f[:, None] * inv_freq[None, :].astype(np.float32)
    cos = np.cos(freqs).astype(np.float32)
    sin = np.sin(freqs).astype(np.float32)
    cs = np.concatenate([cos, cos], axis=1).astype(bf16)  # [8192, 64]
    sn = np.concatenate([sin, sin], axis=1).astype(bf16)

    hT_full = np.ascontiguousarray(hidden_states.astype(np.float32).T)

    in_maps = []
    for c in range(NCORES):
        sl = slice(c * BC, (c + 1) * BC)
        hT = np.ascontiguousarray(hT_full[:, sl])  # [4096, 1024]
        h8 = (hT * np.float32(FSCL)).reshape(C8, 2, P, BC).transpose(0, 2, 1, 3)
        h8 = np.ascontiguousarray(h8).astype(fp8)
        hv = hT.reshape(KC, P, 2, 512).transpose(2, 0, 1, 3)
        hv = np.ascontiguousarray(hv).astype(bf16)
        in_maps.append({
            "hT8": h8,
            "w8": w8_np,
            "hTv": hv,
            "wv": wv_np,
            "wot": wot_np,
            "csb": np.ascontiguousarray(cs[sl].reshape(TT, P, ROT)),
            "snb": np.ascontiguousarray(sn[sl].reshape(TT, P, ROT)),
        })

    res = run_bass_kernel_spmd(nc, in_maps, core_ids=list(range(NCORES)),
                               **_CACHE.get("run_kwargs", {}))
    _CACHE["last_result"] = res
    return np.concatenate(
        [np.ascontiguousarray(
            res.results[c]["outT"].astype(np.float32).T)
         for c in range(NCORES)],
        axis=0)
